# revision 1
# baseline (speedup 1.0000x reference)
"""AutoCorrelation kernel for Trainium2 (Bass/Tile), 8-core data parallel.

Math: the reference computes rfft over the zero-padded head dim (D=64 -> L=512),
multiplies conj(Q)*K, irffts, then MEANS over heads AND the whole lag axis.
Summing a circular correlation over all lags factorizes:
    sum_t corr[t] = (sum_d q[d]) * (sum_d k[d])
so  x_corr_mean[b,l] = 1/(H*L) * sum_h (sum_d q[b,l,h,:]) * (sum_d k[b,l,h,:]).
Then top-6 over l per batch, softmax, weighted sum of values rows -> [B,H,D].

Sharding: batch 16 -> 2 per core across 8 cores, no cross-core communication.

Per core: per batch, q/k row-sums + per-head products on DVE while HWDGE DMAs
stream in (k split 3+1 chunks so the last reduce is short); PE-transpose +
ACT copy + small DMA rake corr into corr2[b, :] (all off-DVE so batch 0's
stretch doesn't contend with batch 1's reduces).  Tail: one MAX8/FIND_INDEX8
pair for both batches, per-batch indirect gathers fed straight from the
FIND_INDEX8 output row (batch base via element_offset), softmax weights
stream-transposed once, per-batch tiny matmuls, stores.
"""

import numpy as np

import concourse.bass as bass
import concourse.mybir as mybir
import concourse.tile as tile
from concourse.masks import make_identity
from concourse.bass_utils import run_bass_kernel_spmd

B, L, H, D = 16, 512, 8, 64
HD = H * D                  # 512
NCORES = 8
BPC = B // NCORES           # 2 batches per core
ROWS = BPC * L              # 1024 rows of [HD] per core
P = 128
NT = ROWS // P              # 8 chunks of 128 rows
TPB = L // P                # 4 chunks per batch
KTOP = 6                    # k = int(log(512)) = 6
SCALE = 1.0 / (H * L)

_CACHE = {}


def _emit(tc, q, k, v, out):
    # out: single [BPC, HD] DRAM AP.
    nc = tc.nc
    from contextlib import ExitStack

    with ExitStack() as ctx:
        main = ctx.enter_context(tc.tile_pool(name="main", bufs=2))
        small = ctx.enter_context(tc.tile_pool(name="small", bufs=1))
        psum = ctx.enter_context(tc.tile_pool(name="psum", bufs=1, space="PSUM"))

        ident = small.tile([P, P], mybir.dt.float32)
        make_identity(nc, ident[:])

        # Per-partition batch masks m0=[1,0], m1=[0,1] (built once, early;
        # 0/1 are exact in f32 so iota can emit float directly).
        m1f = small.tile([BPC, 1], mybir.dt.float32)
        nc.gpsimd.iota(
            m1f[:], pattern=[[0, 1]], base=0, channel_multiplier=1,
            allow_small_or_imprecise_dtypes=True,
        )
        m0f = small.tile([BPC, 1], mybir.dt.float32)
        nc.gpsimd.iota(
            m0f[:], pattern=[[0, 1]], base=1, channel_multiplier=-1,
            allow_small_or_imprecise_dtypes=True,
        )
        mLf = small.tile([BPC, 1], mybir.dt.float32)
        nc.vector.tensor_scalar_mul(mLf[:], m1f[:], float(L))

        q3 = q.rearrange("(t p) m -> t p m", p=P)
        k3 = k.rearrange("(t p) m -> t p m", p=P)

        corr2 = small.tile([BPC, L], mybir.dt.float32)
        for b in range(BPC):
            t0 = b * TPB

            # q split 2+2 chunks, k split 2+1+1: reduces pipeline against
            # DMA arrival and the serial work after the last byte lands is
            # one short 0.25MB reduce instead of a monolithic 1MB one.
            qt = main.tile([P, TPB, HD], mybir.dt.float32, tag=f"qt{b}")
            kt = main.tile([P, TPB, HD], mybir.dt.float32, tag=f"kt{b}")
            q_pieces = [(0, 2), (2, 4)]
            k_pieces = [(0, 2), (2, 3), (3, 4)]
            for lo, hi in q_pieces:
                nc.sync.dma_start(
                    out=qt[:, lo:hi, :],
                    in_=q3[t0 + lo : t0 + hi].rearrange("t p m -> p t m"),
                )
            for lo, hi in k_pieces:
                nc.sync.dma_start(
                    out=kt[:, lo:hi, :],
                    in_=k3[t0 + lo : t0 + hi].rearrange("t p m -> p t m"),
                )
            sq = small.tile([P, TPB * H], mybir.dt.float32, tag=f"sq{b}")
            sk = small.tile([P, TPB * H], mybir.dt.float32, tag=f"sk{b}")
            for lo, hi in q_pieces:
                nc.vector.reduce_sum(
                    out=sq[:, lo * H : hi * H],
                    in_=qt[:, lo:hi, :].rearrange("p t (h d) -> p (t h) d", d=D),
                    axis=mybir.AxisListType.X,
                )
            for lo, hi in k_pieces:
                nc.vector.reduce_sum(
                    out=sk[:, lo * H : hi * H],
                    in_=kt[:, lo:hi, :].rearrange("p t (h d) -> p (t h) d", d=D),
                    axis=mybir.AxisListType.X,
                )
            prod = small.tile([P, TPB * H], mybir.dt.float32, tag=f"prod{b}")
            nc.vector.tensor_mul(prod[:], sq[:], sk[:])
            corr_b = small.tile([P, TPB], mybir.dt.float32, tag=f"corr{b}")
            nc.vector.reduce_sum(
                out=corr_b[:],
                in_=prod[:].rearrange("p (t h) -> p t h", h=H),
                axis=mybir.AxisListType.X,
            )

            # corr [128, 4] -> [4, 128] on PE, ACT copies it out of PSUM,
            # one small DMA rakes it into corr2[b, :].
            psumT = psum.tile([TPB, P], mybir.dt.float32, tag=f"psumT{b}")
            nc.tensor.transpose(out=psumT[:], in_=corr_b[:], identity=ident[:])
            corrT = small.tile([TPB, P], mybir.dt.float32, tag=f"corrT{b}")
            nc.scalar.copy(corrT[:], psumT[:])
            nc.sync.dma_start(out=corr2[b : b + 1, :], in_=corrT[:])

        # ---- tail ----
        maxv = small.tile([BPC, 8], mybir.dt.float32)
        maxi = small.tile([BPC, 8], mybir.dt.uint32)
        nc.vector.max(out=maxv[:], in_=corr2[:])
        nc.vector.max_index(out=maxi[:], in_max=maxv[:], in_values=corr2[:])

        # Combined 12-row gather index column (float staging; indices are
        # exact below 2^24): stage_i row 0 cols 0:6 = idx_b0, row 1 cols
        # 6:12 = idx_b1 + L via the masks; after the 32x32 transpose,
        # col0+col1 rows 0:12 = [idx0, idx1 + L] -> one indirect gather.
        idxf = small.tile([BPC, 8], mybir.dt.float32)
        nc.vector.tensor_copy(idxf[:], maxi[:])
        stage_i = small.tile([32, 32], mybir.dt.float32)
        nc.vector.memset(stage_i[:], 0.0)
        nc.vector.tensor_scalar(
            stage_i[0:BPC, 0:KTOP],
            idxf[:, 0:KTOP],
            m0f[:, 0:1],
            scalar2=None,
            op0=mybir.AluOpType.mult,
        )
        nc.vector.tensor_scalar(
            stage_i[0:BPC, KTOP : 2 * KTOP],
            idxf[:, 0:KTOP],
            mLf[:, 0:1],
            scalar2=m1f[:, 0:1],
            op0=mybir.AluOpType.add,
            op1=mybir.AluOpType.mult,
        )
        stageT_i = small.tile([32, 32], mybir.dt.float32)
        nc.vector.transpose(out=stageT_i[:], in_=stage_i[:])
        combf = small.tile([32, 1], mybir.dt.float32)
        nc.vector.tensor_add(combf[:], stageT_i[:, 0:1], stageT_i[:, 1:2])
        comb = small.tile([32, 1], mybir.dt.uint32)
        nc.vector.tensor_copy(comb[:], combf[:])

        # softmax over the top-6 of corr*SCALE (|corr*SCALE| < ~1, so
        # skipping the max-subtraction is safe in fp32); weights hop onto
        # partitions via one 32x32 stream transpose.
        e = small.tile([BPC, KTOP], mybir.dt.float32)
        nc.scalar.activation(
            out=e[:],
            in_=maxv[:, 0:KTOP],
            func=mybir.ActivationFunctionType.Exp,
            scale=SCALE,
        )
        s = small.tile([BPC, 1], mybir.dt.float32)
        nc.vector.reduce_sum(out=s[:], in_=e[:], axis=mybir.AxisListType.X)
        rs = small.tile([BPC, 1], mybir.dt.float32)
        nc.vector.reciprocal(out=rs[:], in_=s[:])
        w = small.tile([BPC, KTOP], mybir.dt.float32)
        nc.vector.tensor_scalar_mul(w[:], e[:], rs[:, 0:1])

        # Block-diagonal [12, 2] weights via the mask trick: stage_w[b, 0:6]
        # = w_b0 masked to row 0, stage_w[b, 6:12] = w_b1 masked to row 1;
        # transpose -> stageT_w[0:6, 0] = w0, stageT_w[6:12, 1] = w1.
        stage_w = small.tile([32, 32], mybir.dt.float32)
        nc.vector.memset(stage_w[:], 0.0)
        nc.vector.tensor_scalar(
            stage_w[0:BPC, 0:KTOP],
            w[:],
            m0f[:, 0:1],
            scalar2=None,
            op0=mybir.AluOpType.mult,
        )
        nc.vector.tensor_scalar(
            stage_w[0:BPC, KTOP : 2 * KTOP],
            w[:],
            m1f[:, 0:1],
            scalar2=None,
            op0=mybir.AluOpType.mult,
        )
        stageT_w = small.tile([32, 32], mybir.dt.float32)
        nc.vector.transpose(out=stageT_w[:], in_=stage_w[:])

        # ONE indirect gather of all 12 selected value rows, then weighted-sum
        # both batches with ONE matmul, one copy, one store.
        gath = small.tile([2 * KTOP, HD], mybir.dt.float32)
        nc.gpsimd.indirect_dma_start(
            out=gath[:],
            out_offset=None,
            in_=v,
            in_offset=bass.IndirectOffsetOnAxis(
                ap=comb[0 : 2 * KTOP, 0:1], axis=0
            ),
        )
        acc = psum.tile([BPC, HD], mybir.dt.float32)
        nc.tensor.matmul(
            out=acc[:],
            lhsT=stageT_w[0 : 2 * KTOP, 0:BPC],
            rhs=gath[:],
            start=True,
            stop=True,
        )
        outt = small.tile([BPC, HD], mybir.dt.float32)
        nc.scalar.copy(outt[:], acc[:])
        nc.sync.dma_start(out=out, in_=outt[:])


def _build_bass():
    import concourse.bacc as bacc

    nc = bacc.Bacc(trn_type="TRN2", target_bir_lowering=False, debug=False)
    q = nc.dram_tensor("q", [ROWS, HD], mybir.dt.float32, kind="ExternalInput").ap()
    k = nc.dram_tensor("k", [ROWS, HD], mybir.dt.float32, kind="ExternalInput").ap()
    v = nc.dram_tensor("v", [ROWS, HD], mybir.dt.float32, kind="ExternalInput").ap()
    out = nc.dram_tensor(
        "out", [BPC, HD], mybir.dt.float32, kind="ExternalOutput"
    ).ap()
    with tile.TileContext(nc) as tc:
        _emit(tc, q, k, v, out)
    nc.compile()
    return nc


def _get_nc():
    if "nc" not in _CACHE:
        _CACHE["nc"] = _build_bass()
    return _CACHE["nc"]


def run_sharded(queries, keys, values, trace=False, **kw):
    """Shard over 8 cores, run, gather. Returns (out [16,8,64], BassKernelResults)."""
    nc = _get_nc()
    q = np.ascontiguousarray(np.asarray(queries, dtype=np.float32))
    k = np.ascontiguousarray(np.asarray(keys, dtype=np.float32))
    v = np.ascontiguousarray(np.asarray(values, dtype=np.float32))
    in_maps = []
    for c in range(NCORES):
        sl = slice(c * BPC, (c + 1) * BPC)
        in_maps.append(
            {
                "q": q[sl].reshape(ROWS, HD),
                "k": k[sl].reshape(ROWS, HD),
                "v": v[sl].reshape(ROWS, HD),
            }
        )
    res = run_bass_kernel_spmd(nc, in_maps, list(range(NCORES)), trace=trace, **kw)
    out = np.empty((B, H, D), dtype=np.float32)
    for c in range(NCORES):
        out[c * BPC : (c + 1) * BPC] = res.results[c]["out"].reshape(BPC, H, D)
    return out, res


def kernel(queries, keys, values, B=None, **_ignored):
    out, _ = run_sharded(queries, keys, values, trace=False)
    return out



# revision 18
# speedup vs baseline: 1.0922x; 1.0922x over previous
"""AutoCorrelation kernel for Trainium2 (Bass/Tile), 8-core data parallel.

Math: the reference computes rfft over the zero-padded head dim (D=64 -> L=512),
multiplies conj(Q)*K, irffts, then MEANS over heads AND the whole lag axis.
Summing a circular correlation over all lags factorizes:
    sum_t corr[t] = (sum_d q[d]) * (sum_d k[d])
so  x_corr_mean[b,l] = 1/(H*L) * sum_h (sum_d q[b,l,h,:]) * (sum_d k[b,l,h,:]).
Then top-6 over l per batch, softmax, weighted sum of values rows -> [B,H,D].

Sharding: batch 16 -> 2 per core across 8 cores, no cross-core communication.

Per core, two fully per-batch pipelines, staggered so batch 0's entire tail
(top-k, gather, matmul, store) hides under batch 1's loads/reduces:
 - q pieces stream via the ACT-engine HWDGE queue, k via SP (parallel issue),
   [2,1,1] chunk split per (batch, tensor) so the last reduce is short.
 - DVE: d-axis row-sum reduces, per-chunk sq*sk product + h-axis reduce.
 - Each corr chunk column [128,1] is PE-transposed straight into a per-batch
   PSUM row (no SBUF rake DMA).
 - Split top-k: MAX8+FIND_INDEX8 over lag chunks 0-2 runs as soon as those
   chunks land; only chunk 3's is serial. Candidate (value, lag) pairs are
   packed into single fp32s by stuffing the 9-bit lag into the low mantissa
   bits, so the merge is ONE MAX8 over [1,16] and gather indices pop out
   with a bitwise AND -- no full-row FIND_INDEX8 rescan.
 - Per-batch indirect gather of 6 value rows, softmax weights transposed
   onto partitions, single-pass fp32r matmul into PSUM, DVE copy, store.
"""

import numpy as np

import concourse.bass as bass
import concourse.mybir as mybir
import concourse.tile as tile
from concourse.masks import make_identity
from concourse.bass_utils import run_bass_kernel_spmd

B, L, H, D = 16, 512, 8, 64
HD = H * D                  # 512
NCORES = 8
BPC = B // NCORES           # 2 batches per core
ROWS = BPC * L              # 1024 rows of [HD] per core
P = 128
TPB = L // P                # 4 chunks per batch
KTOP = 6                    # k = int(log(512)) = 6
SCALE = 1.0 / (H * L)

MASK_HI = 0xFFFFFE00        # clears the low 9 mantissa bits
MASK_LAG = 0x1FF

_CACHE = {}


def _emit(tc, q, k, v, out):
    nc = tc.nc
    from contextlib import ExitStack

    f32 = mybir.dt.float32
    u32 = mybir.dt.uint32
    f32r = mybir.dt.float32r
    AX = mybir.AxisListType.X
    AluOp = mybir.AluOpType

    with ExitStack() as ctx:
        main = ctx.enter_context(tc.tile_pool(name="main", bufs=1))
        psum = ctx.enter_context(tc.tile_pool(name="psum", bufs=1, space="PSUM"))

        # ---- constants (gpsimd/early, off the critical path) ----
        ident = main.tile([P, P], f32)
        make_identity(nc, ident[:])
        maskc8 = main.tile([1, 8], u32)
        nc.gpsimd.memset(maskc8[:], MASK_HI)
        c1ff6 = main.tile([1, KTOP], u32)
        nc.gpsimd.memset(c1ff6[:], MASK_LAG)
        or384 = main.tile([1, 8], u32)
        nc.gpsimd.memset(or384[:], 3 * P)
        or512 = main.tile([1, KTOP], u32)
        nc.gpsimd.memset(or512[:], L)
        c1ffcol = main.tile([KTOP, 1], u32)
        nc.gpsimd.memset(c1ffcol[:], MASK_LAG)
        or512col = main.tile([KTOP, 1], u32)
        nc.gpsimd.memset(or512col[:], L)
        mstages = {}
        for b in range(BPC):
            m_ = main.tile([32, 32], f32, tag=f"mstage{b}")
            nc.gpsimd.memset(m_[:], 0.0)
            mstages[b] = m_

        q3 = q.rearrange("(t p) m -> t p m", p=P)
        k3 = k.rearrange("(t p) m -> t p m", p=P)

        # ---- loads: q pieces on the ACT HWDGE queue, k pieces on SP ----
        qt, kt = {}, {}
        for b in range(BPC):
            t0 = b * TPB
            qb_big = main.tile([P, 2, HD], f32, tag=f"qb{b}")
            nc.scalar.dma_start(
                out=qb_big[:], in_=q3[t0 : t0 + 2].rearrange("t p m -> p t m")
            )
            qb_c2 = main.tile([P, HD], f32, tag=f"qc2_{b}")
            nc.scalar.dma_start(out=qb_c2[:], in_=q3[t0 + 2])
            qb_c3 = main.tile([P, HD], f32, tag=f"qc3_{b}")
            nc.scalar.dma_start(out=qb_c3[:], in_=q3[t0 + 3])
            qt[b] = (qb_big, qb_c2, qb_c3)

            kb_big = main.tile([P, 2, HD], f32, tag=f"kb{b}")
            nc.sync.dma_start(
                out=kb_big[:], in_=k3[t0 : t0 + 2].rearrange("t p m -> p t m")
            )
            kb_c2 = main.tile([P, HD], f32, tag=f"kc2_{b}")
            nc.sync.dma_start(out=kb_c2[:], in_=k3[t0 + 2])
            kb_c3 = main.tile([P, HD], f32, tag=f"kc3_{b}")
            nc.sync.dma_start(out=kb_c3[:], in_=k3[t0 + 3])
            kt[b] = (kb_big, kb_c2, kb_c3)

        # per-batch state carried between pipeline stages
        st = {}

        def reduces(b, mid=None):
            """DVE row-sums + per-chunk corr columns; PE transposes into the
            per-batch PSUM corr row. `mid` is emitted after chunk 2 so ops
            that only need lag chunks 0-2 slot in before the last reduces."""
            qb_big, qb_c2, qb_c3 = qt[b]
            kb_big, kb_c2, kb_c3 = kt[b]
            sq = main.tile([P, TPB * H], f32, tag=f"sq{b}")
            sk = main.tile([P, TPB * H], f32, tag=f"sk{b}")
            prod = main.tile([P, TPB * H], f32, tag=f"prod{b}")
            corr = main.tile([P, TPB], f32, tag=f"corr{b}")
            psumRow = psum.tile([1, L], f32, tag=f"psumRow{b}")

            def chunk_tail(c):
                nc.vector.tensor_mul(
                    prod[:, c * H : (c + 1) * H],
                    sq[:, c * H : (c + 1) * H],
                    sk[:, c * H : (c + 1) * H],
                )
                nc.vector.reduce_sum(
                    out=corr[:, c : c + 1],
                    in_=prod[:, c * H : (c + 1) * H],
                    axis=AX,
                )
                nc.tensor.transpose(
                    out=psumRow[0:1, c * P : (c + 1) * P],
                    in_=corr[:, c : c + 1],
                    identity=ident[:],
                )

            nc.vector.reduce_sum(
                out=sq[:, 0 : 2 * H],
                in_=qb_big[:].rearrange("p t (h d) -> p (t h) d", d=D),
                axis=AX,
            )
            nc.vector.reduce_sum(
                out=sk[:, 0 : 2 * H],
                in_=kb_big[:].rearrange("p t (h d) -> p (t h) d", d=D),
                axis=AX,
            )
            chunk_tail(0)
            chunk_tail(1)
            for c, qp, kp in ((2, qb_c2, kb_c2), (3, qb_c3, kb_c3)):
                nc.vector.reduce_sum(
                    out=sq[:, c * H : (c + 1) * H],
                    in_=qp[:].rearrange("p (h d) -> p h d", d=D),
                    axis=AX,
                )
                nc.vector.reduce_sum(
                    out=sk[:, c * H : (c + 1) * H],
                    in_=kp[:].rearrange("p (h d) -> p h d", d=D),
                    axis=AX,
                )
                chunk_tail(c)
            st[b] = {"psumRow": psumRow}

        def embed(cand_u_slice, vals, idx, extra_or=None):
            """cand = (vals & MASK_HI) [| 384] | idx  (lag into low mantissa)."""
            nc.vector.tensor_tensor(
                out=cand_u_slice,
                in0=vals[:].bitcast(u32),
                in1=maskc8[:],
                op=AluOp.bitwise_and,
            )
            if extra_or is not None:
                nc.vector.tensor_tensor(
                    out=cand_u_slice,
                    in0=cand_u_slice,
                    in1=extra_or[:],
                    op=AluOp.bitwise_or,
                )
            nc.vector.tensor_tensor(
                out=cand_u_slice,
                in0=cand_u_slice,
                in1=idx[:],
                op=AluOp.bitwise_or,
            )

        def topk_a(b):
            """top-8 of lag chunks 0-2 (runs while chunk 3 is in flight)."""
            psumRow = st[b]["psumRow"]
            row = main.tile([1, L], f32, tag=f"row{b}")
            nc.vector.tensor_copy(row[:, 0 : 3 * P], psumRow[:, 0 : 3 * P])
            cand = main.tile([1, 16], f32, tag=f"cand{b}")
            maxA = main.tile([1, 8], f32, tag=f"maxA{b}")
            idxA = main.tile([1, 8], u32, tag=f"idxA{b}")
            nc.vector.max(out=maxA[:], in_=row[:, 0 : 3 * P])
            nc.vector.max_index(
                out=idxA[:], in_max=maxA[:], in_values=row[:, 0 : 3 * P]
            )
            embed(cand[:].bitcast(u32)[:, 0:8], maxA, idxA)
            st[b]["cand"] = cand
            st[b]["row"] = row

        def topk_b_and_gather(b):
            """chunk-3 top-8, merge, extract row ids, launch the gather."""
            psumRow = st[b]["psumRow"]
            row = st[b]["row"]
            cand = st[b]["cand"]
            nc.vector.tensor_copy(row[:, 3 * P : L], psumRow[:, 3 * P : L])
            maxB = main.tile([1, 8], f32, tag=f"maxB{b}")
            idxB = main.tile([1, 8], u32, tag=f"idxB{b}")
            nc.vector.max(out=maxB[:], in_=row[:, 3 * P : L])
            nc.vector.max_index(
                out=idxB[:], in_max=maxB[:], in_values=row[:, 3 * P : L]
            )
            embed(cand[:].bitcast(u32)[:, 8:16], maxB, idxB, extra_or=or384)

            maxM = main.tile([1, 8], f32, tag=f"maxM{b}")
            nc.vector.max(out=maxM[:], in_=cand[:])
            # hop the embedded top-6 onto partitions (stream transpose), then
            # extract the lag bits column-wise for the gather offsets.
            mstage = mstages[b]
            nc.vector.tensor_copy(mstage[0:1, 0:8], maxM[:])
            mstageT = main.tile([32, 32], f32, tag=f"mstageT{b}")
            nc.vector.transpose(out=mstageT[:], in_=mstage[:])
            comb = main.tile([KTOP, 1], u32, tag=f"comb{b}")
            nc.vector.tensor_tensor(
                out=comb[:],
                in0=mstageT[0:KTOP, 0:1].bitcast(u32),
                in1=c1ffcol[:],
                op=AluOp.bitwise_and,
            )
            if b > 0:
                nc.vector.tensor_tensor(
                    out=comb[:],
                    in0=comb[:],
                    in1=or512col[:],
                    op=AluOp.bitwise_or,
                )
            gath = main.tile([KTOP, HD], f32r, tag=f"gath{b}")
            nc.gpsimd.indirect_dma_start(
                out=gath[:],
                out_offset=None,
                in_=v,
                in_offset=bass.IndirectOffsetOnAxis(ap=comb[0:KTOP, 0:1], axis=0),
            )
            st[b]["maxM"] = maxM
            st[b]["gath"] = gath

        def weights(b):
            """softmax weights onto partitions (runs during the gather flight)."""
            maxM = st[b]["maxM"]
            e = main.tile([1, KTOP], f32, tag=f"e{b}")
            nc.scalar.activation(
                out=e[:],
                in_=maxM[:, 0:KTOP],
                func=mybir.ActivationFunctionType.Exp,
                scale=SCALE,
            )
            s = main.tile([1, 1], f32, tag=f"s{b}")
            nc.vector.reduce_sum(out=s[:], in_=e[:], axis=AX)
            rs = main.tile([1, 1], f32, tag=f"rs{b}")
            nc.vector.reciprocal(out=rs[:], in_=s[:])
            stage = main.tile([32, 32], f32, tag=f"stage{b}")
            nc.vector.memset(stage[:], 0.0)
            nc.vector.tensor_scalar_mul(stage[0:1, 0:KTOP], e[:], rs[:, 0:1])
            stageT = main.tile([32, 32], f32, tag=f"stageT{b}")
            nc.vector.transpose(out=stageT[:], in_=stage[:])
            wcol = main.tile([KTOP, 1], f32r, tag=f"wcol{b}")
            nc.vector.tensor_copy(wcol[:], stageT[0:KTOP, 0:1])
            st[b]["wcol"] = wcol

        def matmul_store(b):
            """single-pass fp32r weighted sum, copy out of PSUM, store."""
            gath = st[b]["gath"]
            wcol = st[b]["wcol"]
            acc = psum.tile([1, HD], f32, tag=f"acc{b}")
            nc.tensor.matmul(
                out=acc[:],
                lhsT=wcol[:],
                rhs=gath[:],
                start=True,
                stop=True,
            )
            outt = main.tile([1, HD], f32, tag=f"outt{b}")
            nc.vector.tensor_copy(outt[:], acc[:])
            nc.sync.dma_start(out=out[b : b + 1, :], in_=outt[:])

        # ---- staggered schedule: b0 tail hides under b1 loads/reduces ----
        reduces(0)
        topk_a(0)
        topk_b_and_gather(0)
        weights(0)
        reduces(1)
        matmul_store(0)
        topk_a(1)
        topk_b_and_gather(1)
        weights(1)
        matmul_store(1)


def _build_bass():
    import concourse.bacc as bacc

    nc = bacc.Bacc(trn_type="TRN2", target_bir_lowering=False, debug=False)
    q = nc.dram_tensor("q", [ROWS, HD], mybir.dt.float32, kind="ExternalInput").ap()
    k = nc.dram_tensor("k", [ROWS, HD], mybir.dt.float32, kind="ExternalInput").ap()
    v = nc.dram_tensor("v", [ROWS, HD], mybir.dt.float32, kind="ExternalInput").ap()
    out = nc.dram_tensor(
        "out", [BPC, HD], mybir.dt.float32, kind="ExternalOutput"
    ).ap()
    with tile.TileContext(nc) as tc:
        _emit(tc, q, k, v, out)
    nc.compile()
    return nc


def _get_nc():
    if "nc" not in _CACHE:
        _CACHE["nc"] = _build_bass()
    return _CACHE["nc"]


def run_sharded(queries, keys, values, trace=False, **kw):
    """Shard over 8 cores, run, gather. Returns (out [16,8,64], BassKernelResults)."""
    nc = _get_nc()
    q = np.ascontiguousarray(np.asarray(queries, dtype=np.float32))
    k = np.ascontiguousarray(np.asarray(keys, dtype=np.float32))
    v = np.ascontiguousarray(np.asarray(values, dtype=np.float32))
    in_maps = []
    for c in range(NCORES):
        sl = slice(c * BPC, (c + 1) * BPC)
        in_maps.append(
            {
                "q": q[sl].reshape(ROWS, HD),
                "k": k[sl].reshape(ROWS, HD),
                "v": v[sl].reshape(ROWS, HD),
            }
        )
    res = run_bass_kernel_spmd(nc, in_maps, list(range(NCORES)), trace=trace, **kw)
    out = np.empty((B, H, D), dtype=np.float32)
    for c in range(NCORES):
        out[c * BPC : (c + 1) * BPC] = res.results[c]["out"].reshape(BPC, H, D)
    return out, res


def kernel(queries, keys, values, B=None, **_ignored):
    out, _ = run_sharded(queries, keys, values, trace=False)
    return out


# revision 25
# speedup vs baseline: 1.1894x; 1.0890x over previous
"""AutoCorrelation kernel for Trainium2 (Bass/Tile), 8-core data parallel.

Math: the reference computes rfft over the zero-padded head dim (D=64 -> L=512),
multiplies conj(Q)*K, irffts, then MEANS over heads AND the whole lag axis.
Summing a circular correlation over all lags factorizes:
    sum_t corr[t] = (sum_d q[d]) * (sum_d k[d])
so  x_corr_mean[b,l] = 1/(H*L) * sum_h (sum_d q[b,l,h,:]) * (sum_d k[b,l,h,:]).
Then top-6 over l per batch, softmax, weighted sum of values rows -> [B,H,D].

Sharding: batch 16 -> 2 per core across 8 cores, no cross-core communication.

Per core, two per-batch pipelines staggered so batch 0's tail hides under
batch 1's loads:
 - q pieces stream on the ACT HWDGE queue, k on SP (parallel issue/queues).
 - DVE does ONLY the d-axis row-sum reduces (the engine crunch); the per-chunk
   sq*sk product + h-axis reduction runs fused on GpSimd
   (scalar_tensor_tensor accum_out), and PSUM->SBUF/output copies run on ACT.
 - Each corr chunk column [128,1] is PE-transposed into a per-batch PSUM row;
   MAX8/FIND_INDEX8 read that PSUM row directly.
 - Split top-k: chunks 0-2 searched while chunk 3 is still in flight; (value,
   lag) candidates packed into single fp32s (9-bit lag in the low mantissa),
   merged with one MAX8 over [1,16]; the lag bits pop out with a bitwise AND.
 - One 32x32 stream transpose moves BOTH the exp-weights (cols 0:6) and the
   gather row-ids (cols 8:16, bit-preserved) onto partitions; the indirect
   gather offsets read straight out of the transposed tile.
 - Weighted sum = single-pass fp32r matmul with UNNORMALIZED exp weights;
   the softmax 1/sum is folded into the ACT copy out of PSUM. Stores issue
   from ACT right after the copy.
"""

import numpy as np

import concourse.bass as bass
import concourse.mybir as mybir
import concourse.tile as tile
from concourse.masks import make_identity
from concourse.bass_utils import run_bass_kernel_spmd

B, L, H, D = 16, 512, 8, 64
HD = H * D                  # 512
NCORES = 8
BPC = B // NCORES           # 2 batches per core
ROWS = BPC * L              # 1024 rows of [HD] per core
P = 128
TPB = L // P                # 4 chunks per batch
KTOP = 6                    # k = int(log(512)) = 6
SCALE = 1.0 / (H * L)

MASK_HI = 0xFFFFFE00        # clears the low 9 mantissa bits
MASK_LAG = 0x1FF

_CACHE = {}


def _emit(tc, q, k, v, out):
    nc = tc.nc
    from contextlib import ExitStack

    f32 = mybir.dt.float32
    u32 = mybir.dt.uint32
    f32r = mybir.dt.float32r
    AX = mybir.AxisListType.X
    AluOp = mybir.AluOpType

    with ExitStack() as ctx:
        main = ctx.enter_context(tc.tile_pool(name="main", bufs=1))
        psum = ctx.enter_context(tc.tile_pool(name="psum", bufs=1, space="PSUM"))

        # ---- constants (gpsimd, off the critical path) ----
        ident = main.tile([P, P], f32)
        make_identity(nc, ident[:])
        maskc8 = main.tile([1, 8], u32)
        nc.gpsimd.memset(maskc8[:], MASK_HI)
        c1ff8 = main.tile([1, 8], u32)
        nc.gpsimd.memset(c1ff8[:], MASK_LAG)
        or384 = main.tile([1, 8], u32)
        nc.gpsimd.memset(or384[:], 3 * P)
        or512 = main.tile([1, 8], u32)
        nc.gpsimd.memset(or512[:], L)
        stages = {}
        idstages = {}
        for b in range(BPC):
            s_ = main.tile([32, 32], f32, tag=f"stage{b}")
            nc.gpsimd.memset(s_[:], 0.0)
            stages[b] = s_
            i_ = main.tile([32, 32], f32, tag=f"idstage{b}")
            nc.gpsimd.memset(i_[:], 0.0)
            idstages[b] = i_

        q3 = q.rearrange("(t p) m -> t p m", p=P)
        k3 = k.rearrange("(t p) m -> t p m", p=P)

        # ---- loads: q pieces on the ACT HWDGE queue, k pieces on SP ----
        qt, kt = {}, {}
        for b in range(BPC):
            t0 = b * TPB
            qb_big = main.tile([P, 2, HD], f32, tag=f"qb{b}")
            nc.scalar.dma_start(
                out=qb_big[:], in_=q3[t0 : t0 + 2].rearrange("t p m -> p t m")
            )
            qb_c2 = main.tile([P, HD], f32, tag=f"qc2_{b}")
            nc.scalar.dma_start(out=qb_c2[:], in_=q3[t0 + 2])
            qb_c3 = main.tile([P, HD], f32, tag=f"qc3_{b}")
            nc.scalar.dma_start(out=qb_c3[:], in_=q3[t0 + 3])
            qt[b] = (qb_big, qb_c2, qb_c3)

            kb_big = main.tile([P, 2, HD], f32, tag=f"kb{b}")
            nc.sync.dma_start(
                out=kb_big[:], in_=k3[t0 : t0 + 2].rearrange("t p m -> p t m")
            )
            kb_c2 = main.tile([P, HD], f32, tag=f"kc2_{b}")
            nc.sync.dma_start(out=kb_c2[:], in_=k3[t0 + 2])
            kb_c3 = main.tile([P, HD], f32, tag=f"kc3_{b}")
            nc.sync.dma_start(out=kb_c3[:], in_=k3[t0 + 3])
            kt[b] = (kb_big, kb_c2, kb_c3)

        st = {}
        for b in range(BPC):
            sq = main.tile([P, TPB * H], f32, tag=f"sq{b}")
            sk = main.tile([P, TPB * H], f32, tag=f"sk{b}")
            corr = main.tile([P, TPB], f32, tag=f"corr{b}")
            psumRow = psum.tile([1, L], f32, tag=f"psumRow{b}")
            st[b] = {"sq": sq, "sk": sk, "corr": corr, "psumRow": psumRow}

        def chunk_tail(b, c):
            """fused sq*sk + h-reduce on GpSimd, then PE-transpose the corr
            column into the per-batch PSUM row."""
            s = st[b]
            junk = main.tile([P, H], f32, tag=f"junk{b}_{c}")
            nc.vector.scalar_tensor_tensor(
                out=junk[:],
                in0=s["sq"][:, c * H : (c + 1) * H],
                scalar=1.0,
                in1=s["sk"][:, c * H : (c + 1) * H],
                op0=AluOp.mult,
                op1=AluOp.mult,
                accum_out=s["corr"][:, c : c + 1],
            )
            nc.tensor.transpose(
                out=s["psumRow"][0:1, c * P : (c + 1) * P],
                in_=s["corr"][:, c : c + 1],
                identity=ident[:],
            )

        def reduces_big_c2(b):
            s = st[b]
            qb_big, qb_c2, _ = qt[b]
            kb_big, kb_c2, _ = kt[b]
            nc.vector.reduce_sum(
                out=s["sq"][:, 0 : 2 * H],
                in_=qb_big[:].rearrange("p t (h d) -> p (t h) d", d=D),
                axis=AX,
            )
            nc.vector.reduce_sum(
                out=s["sk"][:, 0 : 2 * H],
                in_=kb_big[:].rearrange("p t (h d) -> p (t h) d", d=D),
                axis=AX,
            )
            chunk_tail(b, 0)
            chunk_tail(b, 1)
            nc.vector.reduce_sum(
                out=s["sq"][:, 2 * H : 3 * H],
                in_=qb_c2[:].rearrange("p (h d) -> p h d", d=D),
                axis=AX,
            )
            nc.vector.reduce_sum(
                out=s["sk"][:, 2 * H : 3 * H],
                in_=kb_c2[:].rearrange("p (h d) -> p h d", d=D),
                axis=AX,
            )
            chunk_tail(b, 2)

        def reduces_c3(b):
            s = st[b]
            _, _, qb_c3 = qt[b]
            _, _, kb_c3 = kt[b]
            nc.vector.reduce_sum(
                out=s["sq"][:, 3 * H : 4 * H],
                in_=qb_c3[:].rearrange("p (h d) -> p h d", d=D),
                axis=AX,
            )
            nc.vector.reduce_sum(
                out=s["sk"][:, 3 * H : 4 * H],
                in_=kb_c3[:].rearrange("p (h d) -> p h d", d=D),
                axis=AX,
            )
            chunk_tail(b, 3)

        def embed(cand_u_slice, vals, idx, extra_or=None):
            """cand = (vals & MASK_HI) [| 384] | idx  (lag into low mantissa)."""
            nc.vector.tensor_tensor(
                out=cand_u_slice,
                in0=vals[:].bitcast(u32),
                in1=maskc8[:],
                op=AluOp.bitwise_and,
            )
            if extra_or is not None:
                nc.vector.tensor_tensor(
                    out=cand_u_slice,
                    in0=cand_u_slice,
                    in1=extra_or[:],
                    op=AluOp.bitwise_or,
                )
            nc.vector.tensor_tensor(
                out=cand_u_slice,
                in0=cand_u_slice,
                in1=idx[:],
                op=AluOp.bitwise_or,
            )

        def topk_a(b):
            """top-8 of lag chunks 0-2 (runs while chunk 3 is in flight)."""
            psumRow = st[b]["psumRow"]
            row = main.tile([1, L], f32, tag=f"row{b}")
            nc.scalar.copy(row[:, 0 : 3 * P], psumRow[:, 0 : 3 * P])
            cand = main.tile([1, 16], f32, tag=f"cand{b}")
            maxA = main.tile([1, 8], f32, tag=f"maxA{b}")
            idxA = main.tile([1, 8], u32, tag=f"idxA{b}")
            nc.vector.max(out=maxA[:], in_=row[:, 0 : 3 * P])
            nc.vector.max_index(
                out=idxA[:], in_max=maxA[:], in_values=row[:, 0 : 3 * P]
            )
            embed(cand[:].bitcast(u32)[:, 0:8], maxA, idxA)
            st[b]["cand"] = cand
            st[b]["row"] = row

        def topk_b_gather(b):
            """chunk-3 top-8, merge, stage weights+row-ids, launch gather."""
            s = st[b]
            psumRow = s["psumRow"]
            row = s["row"]
            cand = s["cand"]
            stage = stages[b]
            nc.scalar.copy(row[:, 3 * P : L], psumRow[:, 3 * P : L])
            maxB = main.tile([1, 8], f32, tag=f"maxB{b}")
            idxB = main.tile([1, 8], u32, tag=f"idxB{b}")
            nc.vector.max(out=maxB[:], in_=row[:, 3 * P : L])
            nc.vector.max_index(
                out=idxB[:], in_max=maxB[:], in_values=row[:, 3 * P : L]
            )
            embed(cand[:].bitcast(u32)[:, 8:16], maxB, idxB, extra_or=or384)

            maxM = main.tile([1, 8], f32, tag=f"maxM{b}")
            nc.vector.max(out=maxM[:], in_=cand[:])

            # row-ids into idstage cols 0:8 (bit-preserved through the
            # stream transpose -> partitions 0:8), raw exp weights into
            # stage cols 0:6 (written by ACT).
            idstage = idstages[b]
            iu = idstage[:].bitcast(u32)
            nc.vector.tensor_tensor(
                out=iu[0:1, 0:8],
                in0=maxM[:].bitcast(u32),
                in1=c1ff8[:],
                op=AluOp.bitwise_and,
            )
            if b > 0:
                nc.vector.tensor_tensor(
                    out=iu[0:1, 0:8],
                    in0=iu[0:1, 0:8],
                    in1=or512[:],
                    op=AluOp.bitwise_or,
                )
            nc.scalar.activation(
                out=stage[0:1, 0:KTOP],
                in_=maxM[:, 0:KTOP],
                func=mybir.ActivationFunctionType.Exp,
                scale=SCALE,
            )
            sm = main.tile([1, 1], f32, tag=f"sm{b}")
            nc.vector.reduce_sum(
                out=sm[:], in_=stage[0:1, 0:KTOP], axis=AX
            )
            rs = main.tile([1, 1], f32, tag=f"rs{b}")
            nc.vector.reciprocal(out=rs[:], in_=sm[:])
            idstageT = main.tile([32, 32], f32, tag=f"idstageT{b}")
            nc.vector.transpose(out=idstageT[:], in_=idstage[:])
            stageT = main.tile([32, 32], f32, tag=f"stageT{b}")
            nc.vector.transpose(out=stageT[:], in_=stage[:])
            wcol = main.tile([KTOP, 1], f32r, tag=f"wcol{b}")
            nc.vector.tensor_copy(wcol[:], stageT[0:KTOP, 0:1])

            gath = main.tile([8, HD], f32r, tag=f"gath{b}")
            nc.gpsimd.indirect_dma_start(
                out=gath[:],
                out_offset=None,
                in_=v,
                in_offset=bass.IndirectOffsetOnAxis(
                    ap=idstageT[:].bitcast(u32)[0:8, 0:1], axis=0
                ),
            )
            st[b]["gath"] = gath
            st[b]["wcol"] = wcol
            st[b]["rs"] = rs

        def matmul_b(b):
            s = st[b]
            acc = psum.tile([1, HD], f32, tag=f"acc{b}")
            nc.tensor.matmul(
                out=acc[:],
                lhsT=s["wcol"][:],
                rhs=s["gath"][0:KTOP, :],
                start=True,
                stop=True,
            )
            s["acc"] = acc

        def copy_store(b):
            s = st[b]
            outt = main.tile([1, HD], f32, tag=f"outt{b}")
            nc.scalar.activation(
                out=outt[:],
                in_=s["acc"][:],
                func=mybir.ActivationFunctionType.Copy,
                scale=s["rs"][:, 0:1],
            )
            nc.scalar.dma_start(out=out[b : b + 1, :], in_=outt[:])

        # ---- staggered schedule ----
        reduces_big_c2(0)
        reduces_c3(0)
        topk_a(0)
        topk_b_gather(0)
        reduces_big_c2(1)
        topk_a(1)
        matmul_b(0)
        copy_store(0)
        reduces_c3(1)
        topk_b_gather(1)
        matmul_b(1)
        copy_store(1)


def _build_bass():
    import concourse.bacc as bacc

    nc = bacc.Bacc(trn_type="TRN2", target_bir_lowering=False, debug=False)
    q = nc.dram_tensor("q", [ROWS, HD], mybir.dt.float32, kind="ExternalInput").ap()
    k = nc.dram_tensor("k", [ROWS, HD], mybir.dt.float32, kind="ExternalInput").ap()
    v = nc.dram_tensor("v", [ROWS, HD], mybir.dt.float32, kind="ExternalInput").ap()
    out = nc.dram_tensor(
        "out", [BPC, HD], mybir.dt.float32, kind="ExternalOutput"
    ).ap()
    with tile.TileContext(nc) as tc:
        _emit(tc, q, k, v, out)
    nc.compile()
    return nc


def _get_nc():
    if "nc" not in _CACHE:
        _CACHE["nc"] = _build_bass()
    return _CACHE["nc"]


def run_sharded(queries, keys, values, trace=False, **kw):
    """Shard over 8 cores, run, gather. Returns (out [16,8,64], BassKernelResults)."""
    nc = _get_nc()
    q = np.ascontiguousarray(np.asarray(queries, dtype=np.float32))
    k = np.ascontiguousarray(np.asarray(keys, dtype=np.float32))
    v = np.ascontiguousarray(np.asarray(values, dtype=np.float32))
    in_maps = []
    for c in range(NCORES):
        sl = slice(c * BPC, (c + 1) * BPC)
        in_maps.append(
            {
                "q": q[sl].reshape(ROWS, HD),
                "k": k[sl].reshape(ROWS, HD),
                "v": v[sl].reshape(ROWS, HD),
            }
        )
    res = run_bass_kernel_spmd(nc, in_maps, list(range(NCORES)), trace=trace, **kw)
    out = np.empty((B, H, D), dtype=np.float32)
    for c in range(NCORES):
        out[c * BPC : (c + 1) * BPC] = res.results[c]["out"].reshape(BPC, H, D)
    return out, res


def kernel(queries, keys, values, B=None, **_ignored):
    out, _ = run_sharded(queries, keys, values, trace=False)
    return out
